# revision 31
# baseline (speedup 1.0000x reference)
"""NVFP4 (E2M1, block-16) dequant matmul on 8 TRN2 NeuronCores.

out[m, n] = sum_k (LUT[x[m,k]] * xs[m,k//16] * gx) * (LUT[w[n,k]] * ws[n,k//16] * gw) + bias[n]

Sharding: tensor-parallel along N (output features): each of the 8 cores gets
1024 output columns (weight/weight_scale/bias rows), x replicated.

Host-side marshaling (format only, no reference arithmetic):
  - int32 codes -> their exact fp4 values stored as bf16 (16-entry LUT, exact)
  - K-axis permutation r = j*B + b  (k = b*16 + j, B = K/16 blocks):
    within a 128-row K-chunk t = 4j + c, partition p holds block b = c*128+p,
    so the per-block scale for SBUF tile [p, (c n)] is exactly
    scaleT[b, n] laid out [p, (c n)] -- scales align elementwise with the
    code tiles and the dequant multiply is a plain same-shape tensor_tensor.
All multiplies (global-scale fold, x dequant, weight dequant, matmul, bias
add) happen on device.
"""

import json
import os
from contextlib import ExitStack

import ml_dtypes
import numpy as np

import concourse.bass as bass
import concourse.mybir as mybir
import concourse.tile as tile
from concourse.bass_utils import run_bass_kernel_spmd


def _split_multi_waits(m: dict) -> dict:
    """This walrus build allows at most one sync-wait command per instruction.
    Hoist extra waits into standalone EventSemaphore instructions issued just
    before the owning instruction on the same engine queue (semantically
    identical: the engine stalls in order)."""
    for fn in m["functions"]:
        for blk in fn["blocks"]:
            new = []
            ctr = 0
            for inst in blk["instructions"]:
                si = inst.get("sync_info")
                waits = (si or {}).get("on_wait") or []
                if len(waits) > 1:
                    for w in waits[:-1]:
                        new.append({
                            "debug": inst.get("debug", 0),
                            "engine": inst["engine"],
                            "ins": [],
                            "outs": [],
                            "name": f"{inst['name']}-hw{ctr}",
                            "opcode": "EventSemaphore",
                            "sync_info": {"on_update": [], "on_wait": [w]},
                        })
                        ctr += 1
                    si["on_wait"] = [waits[-1]]
                new.append(inst)
            blk["instructions"] = new
    return m


class _SplitWaitBass(bass.Bass):
    def to_json_bytes(self) -> bytes:
        m = json.loads(super().to_json_bytes())
        return json.dumps(_split_multi_waits(m)).encode()

BF16 = ml_dtypes.bfloat16
FP4_LUT = np.array(
    [0.0, 0.5, 1.0, 1.5, 2.0, 3.0, 4.0, 6.0,
     -0.0, -0.5, -1.0, -1.5, -2.0, -3.0, -4.0, -6.0],
    dtype=np.float32,
)

M, K, N = 64, 8192, 8192
NCORES = 8
NS = N // NCORES        # 1024 output columns per core
BLOCK = 16
B = K // BLOCK          # 512 scale blocks along K
P = 128                 # partitions
CHUNKS = K // P         # 64 K-chunks
CB = B // P             # 4 scale-chunk columns (c index)
J = BLOCK               # 16 j-groups (one group = CB chunks = 512 rows)

_CACHE: dict = {}


def _build_program() -> bass.Bass:
    nc = _SplitWaitBass("TRN2", target_bir_lowering=False, debug=False,
                        num_devices=NCORES)
    dt = mybir.dt

    # All inputs arrive host-swizzled to [128, X]: row p holds everything
    # partition p will ever read, contiguously, so DMA descriptors are large.
    wvp = nc.dram_tensor("wvp", [P, CHUNKS * NS], dt.float8e4,
                         kind="ExternalInput").ap()
    wst = nc.dram_tensor("wst", [P, CB * NS], dt.bfloat16,
                         kind="ExternalInput").ap()
    xvp = nc.dram_tensor("xvp", [P, CHUNKS * M], dt.bfloat16,
                         kind="ExternalInput").ap()
    xst = nc.dram_tensor("xst", [P, CB * M], dt.float32,
                         kind="ExternalInput").ap()
    gs = nc.dram_tensor("gs", [P, 2], dt.float32, kind="ExternalInput").ap()
    bia = nc.dram_tensor("bia", [1, NS], dt.bfloat16, kind="ExternalInput").ap()
    out = nc.dram_tensor("out", [M, NS], dt.bfloat16, kind="ExternalOutput").ap()

    with tile.TileContext(nc) as tc, ExitStack() as ctx:
        const = ctx.enter_context(tc.tile_pool(name="const", bufs=1))
        wpool = ctx.enter_context(tc.tile_pool(name="wraw", bufs=3))
        hpool = ctx.enter_context(tc.tile_pool(name="wdq", bufs=3))
        ppool = ctx.enter_context(tc.tile_pool(name="acc", bufs=1, space="PSUM"))

        # ---- constants / x-side setup ----
        # Every DMA rides the single SWDGE queue: one ring drains FIFO, so
        # issue order == completion order and the small x-side loads finish
        # before the 16.8 MB weight stream instead of being diluted by
        # packet-level round-robin across rings.
        gt = const.tile([P, 2], dt.float32)
        nc.gpsimd.dma_start(gt[:], gs[:])
        xsT = const.tile([P, CB * M], dt.float32)                # [128, 256]
        nc.gpsimd.dma_start(xsT[:], xst[:])
        xva = const.tile([P, CHUNKS * M], dt.bfloat16)
        nc.gpsimd.dma_start(xva[:], xvp[:])
        bsb = const.tile([1, NS], dt.bfloat16)
        nc.gpsimd.dma_start(bsb[:], bia[:])
        wsT = const.tile([P, CB * NS], dt.bfloat16)              # [128, 4096]
        nc.gpsimd.dma_start(wsT[:], wst[:])

        gcol = const.tile([P, 1], dt.float32)
        nc.vector.tensor_mul(gcol[:], gt[:, 0:1], gt[:, 1:2])

        # xs * (gx*gw), cast to bf16
        xsb = const.tile([P, CB * M], dt.bfloat16)
        nc.vector.tensor_scalar_mul(xsb[:], xsT[:], gcol[:])

        xhat = const.tile([P, CHUNKS * M], dt.bfloat16)
        for r in range(J):
            sl = slice(r * CB * M, (r + 1) * CB * M)
            nc.vector.tensor_mul(xhat[:, sl], xva[:, sl], xsb[:])

        ones = const.tile([1, M], dt.bfloat16)
        nc.vector.memset(ones[:], 1.0)

        psum = ppool.tile([M, NS], dt.float32)                   # 2 banks

        # ---- main loop: one j-group = 4 K-chunks = [512, NS] of weight ----
        for g in range(J):
            wv = wpool.tile([P, CB * NS], dt.bfloat16)
            nc.gpsimd.dma_start(                                 # fp8 -> bf16 cast
                wv[:], wvp[:, g * CB * NS:(g + 1) * CB * NS])
            wh = hpool.tile([P, CB * NS], dt.bfloat16)
            nc.vector.tensor_mul(wh[:], wv[:], wsT[:])
            for c in range(CB):
                t = g * CB + c
                for h in range(2):
                    nc.tensor.matmul(
                        psum[:, h * 512:(h + 1) * 512],
                        xhat[:, t * M:(t + 1) * M],
                        wh[:, c * NS + h * 512: c * NS + (h + 1) * 512],
                        start=(t == 0),
                        stop=False,
                    )
        for h in range(2):
            nc.tensor.matmul(
                psum[:, h * 512:(h + 1) * 512],
                ones[:1, :],
                bsb[:1, h * 512:(h + 1) * 512],
                start=False,
                stop=True,
            )

        osb = const.tile([M, NS], dt.bfloat16)
        nc.scalar.copy(osb[:], psum[:])
        nc.gpsimd.dma_start(out[:], osb[:])

    return nc


def _perm_k(vals_2d: np.ndarray) -> np.ndarray:
    """[R, K] fp values -> [K, R] with K permuted as r = j*B + b."""
    r = vals_2d.shape[0]
    return (
        vals_2d.reshape(r, B, BLOCK).transpose(2, 1, 0).reshape(K, r)
    )


def _swz(rows_2d: np.ndarray, width: int) -> np.ndarray:
    """[n_chunks*128, width] -> [128, n_chunks*width]: row p holds chunk-major
    data for partition p (per-partition-contiguous DMA layout)."""
    n = rows_2d.shape[0] // P
    return np.ascontiguousarray(
        rows_2d.reshape(n, P, width).transpose(1, 0, 2).reshape(P, n * width)
    )


def prepare_in_maps(**inputs) -> list[dict[str, np.ndarray]]:
    x = np.asarray(inputs["x"]).astype(np.int64)
    xs = np.asarray(inputs["x_scale"], dtype=np.float32)
    gx = np.float32(np.asarray(inputs["x_global_scale"]).reshape(-1)[0])
    w = np.asarray(inputs["weight"]).astype(np.int64)
    ws = np.asarray(inputs["weight_scale"], dtype=np.float32)
    gw = np.float32(np.asarray(inputs["weight_global_scale"]).reshape(-1)[0])
    b = np.asarray(inputs["bias"], dtype=np.float32)

    FP8 = ml_dtypes.float8_e4m3
    xvp = _swz(_perm_k(FP4_LUT[x]).astype(BF16), M)                  # [128, 4096]
    xst = _swz(np.ascontiguousarray(xs.T), M)                        # [128, 256]
    gs = np.tile(np.array([[gx, gw]], dtype=np.float32), (P, 1))

    wv = FP4_LUT[w]                                                  # [N, K] f32
    in_maps = []
    for c in range(NCORES):
        sl = slice(c * NS, (c + 1) * NS)
        in_maps.append({
            "wvp": _swz(_perm_k(wv[sl]).astype(FP8), NS),            # [128, 64*NS]
            "wst": _swz(ws[sl].T.astype(BF16), NS),                  # [128, 4*NS]
            "xvp": xvp,
            "xst": xst,
            "gs": gs,
            "bia": np.ascontiguousarray(b[sl].reshape(1, NS)).astype(BF16),
        })
    return in_maps


LAST_RESULTS = None


def kernel(**inputs) -> np.ndarray:
    global LAST_RESULTS
    if "nc" not in _CACHE:
        _CACHE["nc"] = _build_program()
    nc = _CACHE["nc"]

    in_maps = prepare_in_maps(**inputs)
    res = run_bass_kernel_spmd(nc, in_maps, core_ids=list(range(NCORES)))
    LAST_RESULTS = res
    out = np.concatenate([res.results[c]["out"] for c in range(NCORES)], axis=1)
    return out.astype(BF16)


# revision 61
# speedup vs baseline: 1.1754x; 1.1754x over previous
"""NVFP4 (E2M1, block-16) dequant matmul on 8 TRN2 NeuronCores.

out[m, n] = sum_k (LUT[x[m,k]] * xs[m,k//16] * gx) * (LUT[w[n,k]] * ws[n,k//16] * gw) + bias[n]

Sharding: tensor-parallel along N (output features): each of the 8 cores gets
1024 output columns (weight/weight_scale/bias rows), x replicated.

Host-side marshaling (format only, no reference arithmetic):
  - int32 codes -> their exact fp4 values stored as bf16 (16-entry LUT, exact)
  - K-axis permutation r = j*B + b  (k = b*16 + j, B = K/16 blocks):
    within a 128-row K-chunk t = 4j + c, partition p holds block b = c*128+p,
    so the per-block scale for SBUF tile [p, (c n)] is exactly
    scaleT[b, n] laid out [p, (c n)] -- scales align elementwise with the
    code tiles and the dequant multiply is a plain same-shape tensor_tensor.
All multiplies (global-scale fold, x dequant, weight dequant, matmul, bias
add) happen on device.
"""

import json
import os
from contextlib import ExitStack

import ml_dtypes
import numpy as np

import concourse.bass as bass
import concourse.mybir as mybir
import concourse.tile as tile
from concourse.bass_utils import run_bass_kernel_spmd


def _split_multi_waits(m: dict) -> dict:
    """This walrus build allows at most one sync-wait command per instruction.
    Hoist extra waits into standalone EventSemaphore instructions issued just
    before the owning instruction on the same engine queue (semantically
    identical: the engine stalls in order)."""
    for fn in m["functions"]:
        for blk in fn["blocks"]:
            new = []
            ctr = 0
            for inst in blk["instructions"]:
                si = inst.get("sync_info")
                waits = (si or {}).get("on_wait") or []
                if len(waits) > 1:
                    for w in waits[:-1]:
                        new.append({
                            "debug": inst.get("debug", 0),
                            "engine": inst["engine"],
                            "ins": [],
                            "outs": [],
                            "name": f"{inst['name']}-hw{ctr}",
                            "opcode": "EventSemaphore",
                            "sync_info": {"on_update": [], "on_wait": [w]},
                        })
                        ctr += 1
                    si["on_wait"] = [waits[-1]]
                new.append(inst)
            blk["instructions"] = new
    return m


class _SplitWaitBass(bass.Bass):
    def to_json_bytes(self) -> bytes:
        m = json.loads(super().to_json_bytes())
        return json.dumps(_split_multi_waits(m)).encode()

BF16 = ml_dtypes.bfloat16
FP4_LUT = np.array(
    [0.0, 0.5, 1.0, 1.5, 2.0, 3.0, 4.0, 6.0,
     -0.0, -0.5, -1.0, -1.5, -2.0, -3.0, -4.0, -6.0],
    dtype=np.float32,
)

M, K, N = 64, 8192, 8192
NCORES = 8
NS = N // NCORES        # 1024 output columns per core
BLOCK = 16
B = K // BLOCK          # 512 scale blocks along K
P = 128                 # partitions
CHUNKS = K // P         # 64 K-chunks
CB = B // P             # 4 scale-chunk columns (c index)
J = BLOCK               # 16 j-groups (one group = CB chunks = 512 rows)

_CACHE: dict = {}


def _build_program() -> bass.Bass:
    nc = _SplitWaitBass("TRN2", target_bir_lowering=False, debug=False,
                        num_devices=NCORES)
    dt = mybir.dt

    # All inputs arrive host-swizzled to [128, X]: row p holds everything
    # partition p will ever read, contiguously, so DMA descriptors are large.
    wvp = nc.dram_tensor("wvp", [P, CHUNKS * NS], dt.float8e4,
                         kind="ExternalInput").ap()
    wst = nc.dram_tensor("wst", [P, CB * NS], dt.bfloat16,
                         kind="ExternalInput").ap()
    xvp = nc.dram_tensor("xvp", [P, CHUNKS * M], dt.bfloat16,
                         kind="ExternalInput").ap()
    xst = nc.dram_tensor("xst", [P, CB * M], dt.float32,
                         kind="ExternalInput").ap()
    gs = nc.dram_tensor("gs", [P, 2], dt.float32, kind="ExternalInput").ap()
    bia = nc.dram_tensor("bia", [1, NS], dt.bfloat16, kind="ExternalInput").ap()
    out = nc.dram_tensor("out", [M, NS], dt.bfloat16, kind="ExternalOutput").ap()

    with tile.TileContext(nc) as tc, ExitStack() as ctx:
        const = ctx.enter_context(tc.tile_pool(name="const", bufs=1))
        wpool = ctx.enter_context(tc.tile_pool(name="wraw", bufs=4))
        w8pool = ctx.enter_context(tc.tile_pool(name="wraw8", bufs=4))
        cpool = ctx.enter_context(tc.tile_pool(name="wcast", bufs=3))
        hpool = ctx.enter_context(tc.tile_pool(name="wdq", bufs=3))
        ppool = ctx.enter_context(tc.tile_pool(name="acc", bufs=1, space="PSUM"))

        # ---- constants / x-side setup ----
        # Every DMA rides the single SWDGE queue: one ring drains FIFO, so
        # issue order == completion order and the small x-side loads finish
        # before the 16.8 MB weight stream instead of being diluted by
        # packet-level round-robin across rings.
        # Single SWDGE ring drains FIFO: issue order == completion order, so
        # the prologue is ordered by when each tensor is first needed.
        gt = const.tile([P, 2], dt.float32)
        nc.gpsimd.dma_start(gt[:], gs[:])
        xsT = const.tile([P, CB * M], dt.float32)                # [128, 256]
        nc.gpsimd.dma_start(xsT[:], xst[:])
        wsT = const.tile([P, CB * NS], dt.bfloat16)              # [128, 4096]
        nc.gpsimd.dma_start(wsT[:], wst[:])

        ACT_GROUPS = {2, 4, 6, 8, 10, 12}
        raw_tiles: dict = {}
        for g in range(2):
            wv = wpool.tile([P, CB * NS], dt.bfloat16)
            nc.gpsimd.dma_start(
                wv[:], wvp[:, g * CB * NS:(g + 1) * CB * NS])
            raw_tiles[g] = wv

        xva = const.tile([P, CHUNKS * M], dt.bfloat16)
        for q in range(4):
            qs = slice(q * CHUNKS * M // 4, (q + 1) * CHUNKS * M // 4)
            nc.gpsimd.dma_start(xva[:, qs], xvp[:, qs])
        bsb = const.tile([1, NS], dt.bfloat16)
        nc.gpsimd.dma_start(bsb[:], bia[:])

        # weight-group DMAs in stream (g) order; pool slots pace the issue
        for g in range(2, J):
            gsl = slice(g * CB * NS, (g + 1) * CB * NS)
            if g in ACT_GROUPS:
                wv8 = w8pool.tile([P, CB * NS], dt.float8e4)
                nc.gpsimd.dma_start(wv8[:], wvp[:, gsl])         # plain fp8
                raw_tiles[g] = wv8
            else:
                wv = wpool.tile([P, CB * NS], dt.bfloat16)
                nc.gpsimd.dma_start(wv[:], wvp[:, gsl])          # fp8->bf16 cast
                raw_tiles[g] = wv

        gcol = const.tile([P, 1], dt.float32)
        nc.vector.tensor_mul(gcol[:], gt[:, 0:1], gt[:, 1:2])

        # xs * (gx*gw), cast to bf16
        xsb = const.tile([P, CB * M], dt.bfloat16)
        nc.vector.tensor_scalar_mul(xsb[:], xsT[:], gcol[:])

        xhat = const.tile([P, CHUNKS * M], dt.bfloat16)
        for r in range(J):
            sl = slice(r * CB * M, (r + 1) * CB * M)
            nc.vector.tensor_mul(xhat[:, sl], xva[:, sl], xsb[:])

        ones = const.tile([1, M], dt.bfloat16)
        nc.vector.memset(ones[:], 1.0)

        psum = ppool.tile([M, NS], dt.float32)                   # 2 banks

        # ---- main loop: one j-group = 4 K-chunks = [512, NS] of weight ----
        # Most groups: SWDGE cast-DMA lands bf16 directly (2B/elem SBUF
        # writes). ACT_GROUPS land fp8 (1B/elem) and the idle ScalarE does
        # the widening cast, trimming DMA fabric traffic below the DVE pace.
        for g in range(J):
            if g in ACT_GROUPS:
                wv = cpool.tile([P, CB * NS], dt.bfloat16)
                nc.scalar.copy(wv[:], raw_tiles[g][:])           # ACT cast
            else:
                wv = raw_tiles[g]
            wh = hpool.tile([P, CB * NS], dt.bfloat16)
            nc.vector.tensor_mul(wh[:], wv[:], wsT[:])
            for c in range(CB):
                t = g * CB + c
                for h in range(2):
                    nc.tensor.matmul(
                        psum[:, h * 512:(h + 1) * 512],
                        xhat[:, t * M:(t + 1) * M],
                        wh[:, c * NS + h * 512: c * NS + (h + 1) * 512],
                        start=(t == 0),
                        stop=False,
                    )
        for h in range(2):
            nc.tensor.matmul(
                psum[:, h * 512:(h + 1) * 512],
                ones[:1, :],
                bsb[:1, h * 512:(h + 1) * 512],
                start=False,
                stop=True,
            )

        osb = const.tile([M, NS], dt.bfloat16)
        nc.vector.tensor_copy(osb[:], psum[:])
        nc.gpsimd.dma_start(out[:], osb[:])

    return nc


def _perm_k(vals_2d: np.ndarray) -> np.ndarray:
    """[R, K] fp values -> [K, R] with K permuted as r = j*B + b."""
    r = vals_2d.shape[0]
    return (
        vals_2d.reshape(r, B, BLOCK).transpose(2, 1, 0).reshape(K, r)
    )


def _swz(rows_2d: np.ndarray, width: int) -> np.ndarray:
    """[n_chunks*128, width] -> [128, n_chunks*width]: row p holds chunk-major
    data for partition p (per-partition-contiguous DMA layout)."""
    n = rows_2d.shape[0] // P
    return np.ascontiguousarray(
        rows_2d.reshape(n, P, width).transpose(1, 0, 2).reshape(P, n * width)
    )


def prepare_in_maps(**inputs) -> list[dict[str, np.ndarray]]:
    x = np.asarray(inputs["x"]).astype(np.int64)
    xs = np.asarray(inputs["x_scale"], dtype=np.float32)
    gx = np.float32(np.asarray(inputs["x_global_scale"]).reshape(-1)[0])
    w = np.asarray(inputs["weight"]).astype(np.int64)
    ws = np.asarray(inputs["weight_scale"], dtype=np.float32)
    gw = np.float32(np.asarray(inputs["weight_global_scale"]).reshape(-1)[0])
    b = np.asarray(inputs["bias"], dtype=np.float32)

    FP8 = ml_dtypes.float8_e4m3
    xvp = _swz(_perm_k(FP4_LUT[x]).astype(BF16), M)                  # [128, 4096]
    xst = _swz(np.ascontiguousarray(xs.T), M)                        # [128, 256]
    gs = np.tile(np.array([[gx, gw]], dtype=np.float32), (P, 1))

    wv = FP4_LUT[w]                                                  # [N, K] f32
    in_maps = []
    for c in range(NCORES):
        sl = slice(c * NS, (c + 1) * NS)
        in_maps.append({
            "wvp": _swz(_perm_k(wv[sl]).astype(FP8), NS),            # [128, 64*NS]
            "wst": _swz(ws[sl].T.astype(BF16), NS),                  # [128, 4*NS]
            "xvp": xvp,
            "xst": xst,
            "gs": gs,
            "bia": np.ascontiguousarray(b[sl].reshape(1, NS)).astype(BF16),
        })
    return in_maps


LAST_RESULTS = None


def kernel(**inputs) -> np.ndarray:
    global LAST_RESULTS
    if "nc" not in _CACHE:
        _CACHE["nc"] = _build_program()
    nc = _CACHE["nc"]

    in_maps = prepare_in_maps(**inputs)
    res = run_bass_kernel_spmd(nc, in_maps, core_ids=list(range(NCORES)))
    LAST_RESULTS = res
    out = np.concatenate([res.results[c]["out"] for c in range(NCORES)], axis=1)
    return out.astype(BF16)


# revision 64
# speedup vs baseline: 1.1983x; 1.0195x over previous
"""NVFP4 (E2M1, block-16) dequant matmul on 8 TRN2 NeuronCores.

out[m, n] = sum_k (LUT[x[m,k]] * xs[m,k//16] * gx) * (LUT[w[n,k]] * ws[n,k//16] * gw) + bias[n]

Sharding: tensor-parallel along N (output features): each of the 8 cores gets
1024 output columns (weight/weight_scale/bias rows), x replicated.

Host-side marshaling (format only, no reference arithmetic):
  - int32 codes -> their exact fp4 values stored as bf16 (16-entry LUT, exact)
  - K-axis permutation r = j*B + b  (k = b*16 + j, B = K/16 blocks):
    within a 128-row K-chunk t = 4j + c, partition p holds block b = c*128+p,
    so the per-block scale for SBUF tile [p, (c n)] is exactly
    scaleT[b, n] laid out [p, (c n)] -- scales align elementwise with the
    code tiles and the dequant multiply is a plain same-shape tensor_tensor.
All multiplies (global-scale fold, x dequant, weight dequant, matmul, bias
add) happen on device.
"""

import json
import os
from contextlib import ExitStack

import ml_dtypes
import numpy as np

import concourse.bass as bass
import concourse.mybir as mybir
import concourse.tile as tile
from concourse.bass_utils import run_bass_kernel_spmd


def _split_multi_waits(m: dict) -> dict:
    """This walrus build allows at most one sync-wait command per instruction.
    Hoist extra waits into standalone EventSemaphore instructions issued just
    before the owning instruction on the same engine queue (semantically
    identical: the engine stalls in order)."""
    for fn in m["functions"]:
        for blk in fn["blocks"]:
            new = []
            ctr = 0
            for inst in blk["instructions"]:
                si = inst.get("sync_info")
                waits = (si or {}).get("on_wait") or []
                if len(waits) > 1:
                    for w in waits[:-1]:
                        new.append({
                            "debug": inst.get("debug", 0),
                            "engine": inst["engine"],
                            "ins": [],
                            "outs": [],
                            "name": f"{inst['name']}-hw{ctr}",
                            "opcode": "EventSemaphore",
                            "sync_info": {"on_update": [], "on_wait": [w]},
                        })
                        ctr += 1
                    si["on_wait"] = [waits[-1]]
                new.append(inst)
            blk["instructions"] = new
    return m


class _SplitWaitBass(bass.Bass):
    def to_json_bytes(self) -> bytes:
        m = json.loads(super().to_json_bytes())
        return json.dumps(_split_multi_waits(m)).encode()

BF16 = ml_dtypes.bfloat16
FP4_LUT = np.array(
    [0.0, 0.5, 1.0, 1.5, 2.0, 3.0, 4.0, 6.0,
     -0.0, -0.5, -1.0, -1.5, -2.0, -3.0, -4.0, -6.0],
    dtype=np.float32,
)

M, K, N = 64, 8192, 8192
NCORES = 8
NS = N // NCORES        # 1024 output columns per core
BLOCK = 16
B = K // BLOCK          # 512 scale blocks along K
P = 128                 # partitions
CHUNKS = K // P         # 64 K-chunks
CB = B // P             # 4 scale-chunk columns (c index)
J = BLOCK               # 16 j-groups (one group = CB chunks = 512 rows)

_CACHE: dict = {}


def _build_program() -> bass.Bass:
    nc = _SplitWaitBass("TRN2", target_bir_lowering=False, debug=False,
                        num_devices=NCORES)
    dt = mybir.dt

    # All inputs arrive host-swizzled to [128, X]: row p holds everything
    # partition p will ever read, contiguously, so DMA descriptors are large.
    wvp = nc.dram_tensor("wvp", [P, CHUNKS * NS], dt.float8e4,
                         kind="ExternalInput").ap()
    wst = nc.dram_tensor("wst", [P, CB * NS], dt.bfloat16,
                         kind="ExternalInput").ap()
    xvp = nc.dram_tensor("xvp", [P, CHUNKS * M], dt.bfloat16,
                         kind="ExternalInput").ap()
    xst = nc.dram_tensor("xst", [P, CB * M], dt.float32,
                         kind="ExternalInput").ap()
    gs = nc.dram_tensor("gs", [P, 2], dt.float32, kind="ExternalInput").ap()
    bia = nc.dram_tensor("bia", [1, NS], dt.bfloat16, kind="ExternalInput").ap()
    out = nc.dram_tensor("out", [M, NS], dt.bfloat16, kind="ExternalOutput").ap()

    with tile.TileContext(nc) as tc, ExitStack() as ctx:
        const = ctx.enter_context(tc.tile_pool(name="const", bufs=1))
        wpool = ctx.enter_context(tc.tile_pool(name="wraw", bufs=4))
        w8pool = ctx.enter_context(tc.tile_pool(name="wraw8", bufs=4))
        cpool = ctx.enter_context(tc.tile_pool(name="wcast", bufs=3))
        hpool = ctx.enter_context(tc.tile_pool(name="wdq", bufs=3))
        ppool = ctx.enter_context(tc.tile_pool(name="acc", bufs=1, space="PSUM"))

        # ---- constants / x-side setup ----
        # Every DMA rides the single SWDGE queue: one ring drains FIFO, so
        # issue order == completion order and the small x-side loads finish
        # before the 16.8 MB weight stream instead of being diluted by
        # packet-level round-robin across rings.
        # Single SWDGE ring drains FIFO: issue order == completion order, so
        # the prologue is ordered by when each tensor is first needed.
        gt = const.tile([P, 2], dt.float32)
        nc.gpsimd.dma_start(gt[:], gs[:])
        xsT = const.tile([P, CB * M], dt.float32)                # [128, 256]
        nc.gpsimd.dma_start(xsT[:], xst[:])
        wsT = const.tile([P, CB * NS], dt.bfloat16)              # [128, 4096]
        nc.gpsimd.dma_start(wsT[:], wst[:])

        ACT_GROUPS = {2, 4, 6, 8, 10, 12}
        raw_tiles: dict = {}
        for g in range(2):
            wv = wpool.tile([P, CB * NS], dt.bfloat16)
            nc.gpsimd.dma_start(
                wv[:], wvp[:, g * CB * NS:(g + 1) * CB * NS])
            raw_tiles[g] = wv

        xva = const.tile([P, CHUNKS * M], dt.bfloat16)
        for q in range(4):
            qs = slice(q * CHUNKS * M // 4, (q + 1) * CHUNKS * M // 4)
            nc.gpsimd.dma_start(xva[:, qs], xvp[:, qs])
        bsb = const.tile([1, NS], dt.bfloat16)
        nc.gpsimd.dma_start(bsb[:], bia[:])

        # weight-group DMAs in stream (g) order; pool slots pace the issue
        for g in range(2, J):
            gsl = slice(g * CB * NS, (g + 1) * CB * NS)
            if g in ACT_GROUPS:
                wv8 = w8pool.tile([P, CB * NS], dt.float8e4)
                nc.gpsimd.dma_start(wv8[:], wvp[:, gsl])         # plain fp8
                raw_tiles[g] = wv8
            else:
                wv = wpool.tile([P, CB * NS], dt.bfloat16)
                nc.gpsimd.dma_start(wv[:], wvp[:, gsl])          # fp8->bf16 cast
                raw_tiles[g] = wv

        gcol = const.tile([P, 1], dt.float32)
        nc.vector.tensor_mul(gcol[:], gt[:, 0:1], gt[:, 1:2])

        # xs * (gx*gw), cast to bf16
        xsb = const.tile([P, CB * M], dt.bfloat16)
        nc.vector.tensor_scalar_mul(xsb[:], xsT[:], gcol[:])

        # replicate xsb 4x on the idle ScalarE so the x dequant runs as 4
        # wide DVE ops instead of 16 narrow ones
        xsr = const.tile([P, 4 * CB * M], dt.bfloat16)           # [128, 1024]
        for r in range(4):
            nc.scalar.copy(xsr[:, r * CB * M:(r + 1) * CB * M], xsb[:])
        xhat = const.tile([P, CHUNKS * M], dt.bfloat16)
        for r in range(4):
            sl = slice(r * 4 * CB * M, (r + 1) * 4 * CB * M)
            nc.vector.tensor_mul(xhat[:, sl], xva[:, sl], xsr[:])

        ones = const.tile([1, M], dt.bfloat16)
        nc.vector.memset(ones[:], 1.0)

        psum = ppool.tile([M, NS], dt.float32)                   # 2 banks

        # bias rides the FIRST accumulation write (start=True clears PSUM),
        # keeping the kernel tail free of extra matmuls
        for h in range(2):
            nc.tensor.matmul(
                psum[:, h * 512:(h + 1) * 512],
                ones[:1, :],
                bsb[:1, h * 512:(h + 1) * 512],
                start=True,
                stop=False,
            )

        # ---- main loop: one j-group = 4 K-chunks = [512, NS] of weight ----
        # Most groups: SWDGE cast-DMA lands bf16 directly (2B/elem SBUF
        # writes). ACT_GROUPS land fp8 (1B/elem) and the idle ScalarE does
        # the widening cast, trimming DMA fabric traffic below the DVE pace.
        for g in range(J):
            if g in ACT_GROUPS:
                wv = cpool.tile([P, CB * NS], dt.bfloat16)
                nc.scalar.copy(wv[:], raw_tiles[g][:])           # ACT cast
            else:
                wv = raw_tiles[g]
            if g == J - 1:
                # last group per-chunk: only 2 matmuls trail the final TT
                for c in range(CB):
                    t = g * CB + c
                    whc = hpool.tile([P, NS], dt.bfloat16, tag="htail")
                    nc.vector.tensor_mul(
                        whc[:], wv[:, c * NS:(c + 1) * NS],
                        wsT[:, c * NS:(c + 1) * NS])
                    for h in range(2):
                        nc.tensor.matmul(
                            psum[:, h * 512:(h + 1) * 512],
                            xhat[:, t * M:(t + 1) * M],
                            whc[:, h * 512:(h + 1) * 512],
                            start=False,
                            stop=(t == CHUNKS - 1),
                        )
                continue
            wh = hpool.tile([P, CB * NS], dt.bfloat16)
            nc.vector.tensor_mul(wh[:], wv[:], wsT[:])
            for c in range(CB):
                t = g * CB + c
                for h in range(2):
                    nc.tensor.matmul(
                        psum[:, h * 512:(h + 1) * 512],
                        xhat[:, t * M:(t + 1) * M],
                        wh[:, c * NS + h * 512: c * NS + (h + 1) * 512],
                        start=False,
                        stop=(t == CHUNKS - 1),
                    )

        osb = const.tile([M, NS], dt.bfloat16)
        nc.vector.tensor_copy(osb[:], psum[:])
        nc.gpsimd.dma_start(out[:], osb[:])

    return nc


def _perm_k(vals_2d: np.ndarray) -> np.ndarray:
    """[R, K] fp values -> [K, R] with K permuted as r = j*B + b."""
    r = vals_2d.shape[0]
    return (
        vals_2d.reshape(r, B, BLOCK).transpose(2, 1, 0).reshape(K, r)
    )


def _swz(rows_2d: np.ndarray, width: int) -> np.ndarray:
    """[n_chunks*128, width] -> [128, n_chunks*width]: row p holds chunk-major
    data for partition p (per-partition-contiguous DMA layout)."""
    n = rows_2d.shape[0] // P
    return np.ascontiguousarray(
        rows_2d.reshape(n, P, width).transpose(1, 0, 2).reshape(P, n * width)
    )


def prepare_in_maps(**inputs) -> list[dict[str, np.ndarray]]:
    x = np.asarray(inputs["x"]).astype(np.int64)
    xs = np.asarray(inputs["x_scale"], dtype=np.float32)
    gx = np.float32(np.asarray(inputs["x_global_scale"]).reshape(-1)[0])
    w = np.asarray(inputs["weight"]).astype(np.int64)
    ws = np.asarray(inputs["weight_scale"], dtype=np.float32)
    gw = np.float32(np.asarray(inputs["weight_global_scale"]).reshape(-1)[0])
    b = np.asarray(inputs["bias"], dtype=np.float32)

    FP8 = ml_dtypes.float8_e4m3
    xvp = _swz(_perm_k(FP4_LUT[x]).astype(BF16), M)                  # [128, 4096]
    xst = _swz(np.ascontiguousarray(xs.T), M)                        # [128, 256]
    gs = np.tile(np.array([[gx, gw]], dtype=np.float32), (P, 1))

    wv = FP4_LUT[w]                                                  # [N, K] f32
    in_maps = []
    for c in range(NCORES):
        sl = slice(c * NS, (c + 1) * NS)
        in_maps.append({
            "wvp": _swz(_perm_k(wv[sl]).astype(FP8), NS),            # [128, 64*NS]
            "wst": _swz(ws[sl].T.astype(BF16), NS),                  # [128, 4*NS]
            "xvp": xvp,
            "xst": xst,
            "gs": gs,
            "bia": np.ascontiguousarray(b[sl].reshape(1, NS)).astype(BF16),
        })
    return in_maps


LAST_RESULTS = None


def kernel(**inputs) -> np.ndarray:
    global LAST_RESULTS
    if "nc" not in _CACHE:
        _CACHE["nc"] = _build_program()
    nc = _CACHE["nc"]

    in_maps = prepare_in_maps(**inputs)
    res = run_bass_kernel_spmd(nc, in_maps, core_ids=list(range(NCORES)))
    LAST_RESULTS = res
    out = np.concatenate([res.results[c]["out"] for c in range(NCORES)], axis=1)
    return out.astype(BF16)
